# revision 17
# baseline (speedup 1.0000x reference)
"""Trainium2 Bass kernel for nn_CapsNet_69114613730132.

Strategy (8 NeuronCores, SPMD, zero collectives):
  The CapsNet routing loop is degenerate (self.bij is never updated, so
  cij stays 1/512) and collapses to: conv1 -> conv2 -> squash ->
  4096->160 matvec -> elementwise squash. The convolutions are tiny, so
  cross-core collectives (a measured ~75us NRT CC setup gap per
  execution on this axon fabric) cost more than replicating them.

  * Every core computes conv1 + conv2 (PrimaryCaps) + squash redundantly:
      conv1 as a 243-contraction matmul over a host-built im2col of x;
      conv2 as 81 (dy,dx) PSUM-accumulated matmuls over strided views of
      h (no im2col materialization), weights stationary, bf16.
  * The DigitCaps matvec output (160 = 10*16) is sharded 20-per-core via
    per-core weight slices => cores are fully independent; the host just
    concatenates the 8 (1,20) results. No communication at all.
  * All PE compute in bf16 (weights host-cast), f32 PSUM/vector math.

  Schedule (from perfetto trace of the 35.7us baseline):
  - the w2 stream is HBM-bound (8 cores x 5.3MB aggregate ~3TB/s), so
    w2 chunks go FIRST and alone on the sync HWDGE ring; conv1/small
    inputs ride the scalar ring in parallel. Tapered chunk sizes keep
    the post-stream matmul tail tiny.
  - the serial tail (squash -> digitcaps -> squash -> out DMA) is
    restructured: single conv2 PSUM tile + host-built (128,32) bias
    tensor (1 add), Sqrt on Scalar overlapped with 1+sq/recip on
    Vector, final squash reads psum_d twice (Square on Scalar +
    single-PSUM-operand tensor_tensor) instead of copy+mul chains.

kernel(**inputs) takes the FULL unsharded inputs and returns the full
(1,1,10,16,1) float32 output.
"""
import numpy as np
import ml_dtypes

import concourse.bass as bass
import concourse.bacc as bacc
import concourse.tile as tile
import concourse.mybir as mybir
from concourse.bass_utils import run_bass_kernel_spmd
from concourse.tile import ScopedClock, add_dep_helper

FAST_TAIL = True


class FastTailTileContext(tile.TileContext):
    """TileContext tail with a 1-hop handshake instead of the all-engine
    barriers (each an EVSEM polling butterfly measured at ~7us here).

    The sync.drain waits for every tracked semaphore target, so by the
    time it passes, every sem-touching instruction on every engine has
    retired (each engine's last real work is upstream of the output DMA
    the drain waits on). A single drain->GpSimd semaphore hop then orders
    the sem/DMA-state clears; the next execution's NEFF entry barrier
    orders everything else."""

    def _drain_and_barrier(self, tick_clock, wait_clock):
        if not FAST_TAIL:
            return super()._drain_and_barrier(tick_clock, wait_clock)
        nc = self.nc
        # GpSimd (the clearing engine, otherwise idle here) waits on every
        # tracked semaphore's final value itself, then clears.
        drain_inst = nc.gpsimd.drain()
        wait_clock.add_sem_waits(
            drain_inst.ins, ScopedClock({None: tick_clock.global_clock})
        )
        # DMA completion sems update asynchronously and the final DMA (the
        # output store) has no downstream consumer, so the clock misses it:
        # wait each DMA-lane sem out to its summed final value explicitly.
        dma_totals = {}
        for insts in self.ordered_instructions_by_block.values():
            for i in insts:
                si = i.sync_info
                if si is None or not si.on_update:
                    continue
                for u in si.on_update:
                    if (u.sync_type == "semaphore" and u.update_value
                            and (u.ant_name or "").startswith("DMA")):
                        k = (u.id, u.ant_name)
                        dma_totals[k] = dma_totals.get(k, 0) + u.update_value
        # Only GpSimd (which does the range-clear below) needs to wait the
        # DMA totals: the walrus exit barrier that follows the tile block
        # already orders every other engine's postamble sem clears after
        # GpSimd's arrival. Putting these waits on all engines costs ~1us
        # of serial EVENT_SEMAPHORE time on the exit critical path.
        handles = {h.num: h for h in self.sems.allocated().values()}
        for (sid, _), tot in sorted(dma_totals.items()):
            if sid in handles:
                nc.gpsimd.wait_ge(handles[sid], tot)
        popped = nc._tile_sem_poison_stack.pop()
        assert popped is self._sem_poison
        nc.clear_and_free_semaphores(list(self.sems.allocated().values()))

BF16 = ml_dtypes.bfloat16
F32 = mybir.dt.float32
BF = mybir.dt.bfloat16

NCORES = 8
KI = 20             # digitcaps output elems per core (160 = 8*20)
# dydx positions per w2 DMA chunk. The tile framework rotates ~9
# DMA-completion sems GLOBALLY across queues; keeping the kernel's
# total input-DMA count at 9 (5 w2 + 2 c1 + cbb + v) avoids issue
# throttling on sem reuse entirely. The tiny final chunk keeps the
# post-stream matmul tail at 2 matmuls.
W2CHUNKS = [20, 20, 20, 20, 1]
assert sum(W2CHUNKS) == 81


# --------------------------------------------------------------------------
# Host-side input marshalling (pure layout transforms + dtype casts)
# --------------------------------------------------------------------------

def _host_prep(x, conv_w, conv_b, pri_w, pri_b, W):
    x = np.asarray(x, np.float32)
    conv_w = np.asarray(conv_w, np.float32)
    conv_b = np.asarray(conv_b, np.float32)
    pri_w = np.asarray(pri_w, np.float32)
    pri_b = np.asarray(pri_b, np.float32)
    W = np.asarray(W, np.float32)

    # im2col of x: (243, 256), row (c,dy,dx), col (oy*16+ox)
    im2col1 = np.empty((3, 9, 9, 16, 16), np.float32)
    for dy in range(9):
        for dx in range(9):
            im2col1[:, dy, dx] = x[0, :, dy:dy + 16, dx:dx + 16]
    im2col1 = im2col1.reshape(243, 256).astype(BF16)

    W1T = conv_w.reshape(128, 243).T.astype(BF16)  # (243, 128)

    # (ic, dydx*256 + oc2) with oc2 = cap*8 + j
    w2s = (pri_w.reshape(256, 128, 9, 9)
           .transpose(2, 3, 1, 0)          # (dy, dx, ic, oc2)
           .reshape(81, 128, 256)
           .transpose(1, 0, 2)             # (ic, dydx, oc2)
           .reshape(128, 81 * 256).astype(BF16))

    # digitcaps weights V[h, s, p, ki]:
    #   oc2 = 128h+p; cap=oc2>>3; j=oc2&7; n = cap*16 + j*2 + (s>>3); jj = s&7
    Wd = W[0]  # (512, 10, 16, 8)
    oc2 = np.arange(256)
    n_base = (oc2 >> 3) * 16 + (oc2 & 7) * 2
    V = np.empty((2, 16, 128, 160), np.float32)
    for s in range(16):
        sel = Wd[n_base + (s >> 3), :, :, s & 7]      # (256, 10, 16)
        V[:, s] = sel.reshape(2, 128, 160)

    # col 0: conv1 bias; cols 1..32: conv2 bias broadcast to [p, h*16+s]
    # (single merged DMA keeps the kernel at 9 input DMAs = the global
    # DMA-sem rotation depth, so no issue ever throttles on sem reuse).
    cbb = np.empty((128, 33), np.float32)
    cbb[:, 0] = conv_b
    cbb[:, 1:] = np.repeat(pri_b.reshape(2, 128).T, 16, axis=1)
    cbb = np.ascontiguousarray(cbb)

    c1 = np.concatenate([im2col1, W1T], axis=1)  # (243, 384) = [im2col | w1t]
    shared = {
        "c1_a": np.ascontiguousarray(c1[:128]),
        "c1_b": np.ascontiguousarray(c1[128:]),
        "w2s": w2s,
        "cbb": cbb,
    }
    per_core = []
    for c in range(NCORES):
        vsl = V[:, :, :, c * KI:(c + 1) * KI]                     # (2,16,128,20)
        vsl = vsl.transpose(2, 0, 1, 3).reshape(128, 32 * KI)     # (128, 640)
        d = dict(shared)
        d["v"] = np.ascontiguousarray(vsl).astype(BF16)
        per_core.append(d)
    return per_core


INPUT_SPECS = {
    "c1_a": ((128, 384), BF),
    "c1_b": ((115, 384), BF),
    "w2s": ((128, 81 * 256), BF),
    "v": ((128, 32 * KI), BF),
    "cbb": ((128, 33), F32),
}


# --------------------------------------------------------------------------
# Device IR
# --------------------------------------------------------------------------

def emit_kernel(tc, out_ap, ins):
    nc = tc.nc
    with (
        tc.tile_pool(name="sb", bufs=1) as sb,
        tc.tile_pool(name="ps", bufs=1, space="PSUM") as ps,
    ):
        # ---- DMA order (measured): rings share the 16 DMA engines and
        # the sync-ring w2 flood starves everything issued after it, so
        # anything needed early must be FIFO-AHEAD of the flood on the
        # sync ring itself. Chunk 0 goes absolutely first (its data
        # starts flowing while the later issues are still executing),
        # then the small early inputs, then the remaining chunks. Only v
        # (first needed by digitcaps at ~26us) can tolerate riding the
        # starved scalar ring. 8 sync + 1 scalar DMAs = the global ~9-sem
        # rotation, so no issue throttles on sem reuse.
        c1_a_sb = sb.tile([128, 384], BF)
        c1_b_sb = sb.tile([115, 384], BF)
        cbb_sb = sb.tile([128, 33], F32)
        v_sb = sb.tile([128, 32 * KI], BF)
        nc.scalar.dma_start(v_sb[:], ins["v"][:])
        w2t = []
        offs = [0]
        for cn in W2CHUNKS:
            offs.append(offs[-1] + cn)
        for j, cn in enumerate(W2CHUNKS):
            w2t.append(sb.tile([128, cn * 256], BF, name=f"w2t{j}"))
        nc.sync.dma_start(w2t[0][:], ins["w2s"][:, offs[0] * 256:offs[1] * 256])
        for t, name in ((c1_a_sb, "c1_a"), (c1_b_sb, "c1_b"), (cbb_sb, "cbb")):
            nc.sync.dma_start(t[:], ins[name][:])
        for j in range(1, len(W2CHUNKS)):
            nc.sync.dma_start(
                w2t[j][:], ins["w2s"][:, offs[j] * 256:offs[j + 1] * 256])

        # Own zero-bias tile for scalar activations: avoids the framework
        # const-AP SBUF region, which lets build_nc() drop the const-setup
        # all-engine barrier from the main block (~1us of entry latency).
        zb = sb.tile([128, 1], F32)
        nc.vector.memset(zb[:], 0.0)

        # ---- conv1: h = W1T.T @ im2col + conv_b  -> (128, 256) bf16
        psum1 = ps.tile([128, 256], F32)
        nc.tensor.matmul(psum1[:], c1_a_sb[:, 256:384], c1_a_sb[:, 0:256],
                         start=True, stop=False)
        nc.tensor.matmul(psum1[:], c1_b_sb[:, 256:384], c1_b_sb[:, 0:256],
                         start=False, stop=True)
        h_sb = sb.tile([128, 256], BF)
        nc.vector.tensor_scalar_add(h_sb[:], psum1[:], cbb_sb[:, 0:1])
        h4 = h_sb[:].rearrange("p (y x) -> p y x", y=16)

        # ---- conv2: 81 strided-view matmuls per oc2-half, PSUM-accumulated
        # into one (128, 32) tile: cols [0:16] = half 0, [16:32] = half 1.
        psum2 = ps.tile([128, 32], F32)
        # Two interleaved accumulation groups share this tile; a start=True
        # reset from one group stomps the other's first tap, so zero the
        # tile once (early, off the critical path) and accumulate without
        # start resets.
        nc.vector.memset(psum2[:], 0.0)
        chunk_of = []
        for j, cn in enumerate(W2CHUNKS):
            chunk_of += [(j, k) for k in range(cn)]
        for dydx in range(81):
            dy, dx = divmod(dydx, 9)
            j, jj = chunk_of[dydx]
            rhs = h4[:, dy:dy + 8:2, dx:dx + 8:2]
            for hh in range(2):
                nc.tensor.matmul(
                    psum2[:, hh * 16:(hh + 1) * 16],
                    w2t[j][:, jj * 256 + hh * 128: jj * 256 + (hh + 1) * 128],
                    rhs,
                    start=False, stop=(dydx == 80),
                )

        # ---- x2b = psum2 + pri_b (single tensor_tensor add)
        x2b = sb.tile([128, 32], F32)
        nc.vector.tensor_tensor(x2b[:], psum2[:], cbb_sb[:, 1:33],
                                op=mybir.AluOpType.add)

        # ---- squash factors per (p, h, s_hi) group of 8
        t2 = sb.tile([128, 32], F32)
        nc.vector.tensor_mul(t2[:], x2b[:], x2b[:])
        sq = sb.tile([128, 4], F32)
        nc.vector.tensor_reduce(
            sq[:], t2[:].rearrange("p (g e) -> p g e", e=8),
            axis=mybir.AxisListType.X, op=mybir.AluOpType.add,
        )
        # f = sqrt(sq)/512 / (1+sq)   (1/512 cij folded in)
        # (Sqrt on Scalar overlaps the 1/(1+sq) chain on Vector)
        r_ = sb.tile([128, 4], F32)
        nc.scalar.activation(
            r_[:], sq[:], mybir.ActivationFunctionType.Sqrt,
            bias=zb[:], scale=1.0 / (512.0 * 512.0),
        )
        d2 = sb.tile([128, 4], F32)
        nc.vector.tensor_scalar_add(d2[:], sq[:], 1.0)
        rec2 = sb.tile([128, 4], F32)
        nc.vector.reciprocal(rec2[:], d2[:])
        f_ = sb.tile([128, 4], F32)
        nc.vector.tensor_mul(f_[:], r_[:], rec2[:])

        # u in two halves so the digitcaps matmuls for idx 0..15 start
        # while Vector is still producing idx 16..31.
        u_sb = sb.tile([128, 32], BF)
        for hh in range(2):
            nc.vector.tensor_mul(
                u_sb[:, hh * 16:(hh + 1) * 16].rearrange(
                    "p (g e) -> p g e", e=8),
                x2b[:, hh * 16:(hh + 1) * 16].rearrange(
                    "p (g e) -> p g e", e=8),
                f_[:, hh * 2:(hh + 1) * 2].broadcast_to((128, 2, 8)),
            )

        # ---- digitcaps matvec: psum_d[0, ki] = sum_{h,s,p} u * V
        psum_d = ps.tile([1, KI], F32)
        for idx in range(32):
            nc.tensor.matmul(
                psum_d[:],
                u_sb[:, idx:idx + 1],
                v_sb[:, idx * KI:(idx + 1) * KI],
                start=(idx == 0), stop=(idx == 31),
            )

        # ---- final elementwise squash: vij = s*|s|/(1+s^2)
        # psum_d is read twice via single-PSUM-operand ops (no SBUF copy):
        # Scalar squares it; Vector multiplies it by sqrt(t3) later.
        t3 = sb.tile([1, KI], F32)
        nc.scalar.activation(t3[:], psum_d[:],
                             mybir.ActivationFunctionType.Square,
                             bias=zb[0:1, :])
        a3 = sb.tile([1, KI], F32)
        nc.scalar.activation(a3[:], t3[:], mybir.ActivationFunctionType.Sqrt,
                             bias=zb[0:1, :])
        d3 = sb.tile([1, KI], F32)
        nc.vector.tensor_scalar_add(d3[:], t3[:], 1.0)
        rec3 = sb.tile([1, KI], F32)
        nc.vector.reciprocal(rec3[:], d3[:])
        m3 = sb.tile([1, KI], F32)
        nc.vector.tensor_tensor(m3[:], psum_d[:], a3[:],
                                op=mybir.AluOpType.mult)
        o3 = sb.tile([1, KI], F32)
        nc.vector.tensor_mul(o3[:], m3[:], rec3[:])
        nc.sync.dma_start(out_ap[:], o3[:])


# --------------------------------------------------------------------------
# Build + run
# --------------------------------------------------------------------------

_CACHE = {}


def build_nc():
    nc = bacc.Bacc(
        "TRN2", target_bir_lowering=False, debug=False, num_devices=NCORES
    )
    ins = {
        name: nc.dram_tensor(name, list(shape), dt, kind="ExternalInput").ap()
        for name, (shape, dt) in INPUT_SPECS.items()
    }
    out_ap = nc.dram_tensor("out", [1, KI], F32, kind="ExternalOutput").ap()
    # Drop the const-AP setup barrier from the main block: nothing in this
    # kernel reads the framework const APs (activations get an explicit
    # in-block zero-bias tile), so the all-engine rendezvous before the
    # tile-block branch only delays the first DMA issue by ~1us.
    mb = nc.main_func.blocks[0]
    mb.instructions = [
        i for i in mb.instructions
        if not isinstance(i, (mybir.InstDrain, mybir.InstEventSemaphore))
    ]
    with FastTailTileContext(nc) as tc:
        emit_kernel(tc, out_ap, ins)
    nc.compile()
    return nc


def kernel(**inputs):
    per_core = _host_prep(**inputs)
    if "nc" not in _CACHE:
        _CACHE["nc"] = build_nc()
    res = run_bass_kernel_spmd(
        _CACHE["nc"], per_core, core_ids=list(range(NCORES))
    )
    out = np.concatenate(
        [np.asarray(res.results[c]["out"], np.float32).reshape(-1)
         for c in range(NCORES)]
    )
    return out.reshape(1, 1, 10, 16, 1)


# revision 18
# speedup vs baseline: 1.0427x; 1.0427x over previous
"""Trainium2 Bass kernel for nn_CapsNet_69114613730132.

Strategy (8 NeuronCores, SPMD, zero collectives):
  The CapsNet routing loop is degenerate (self.bij is never updated, so
  cij stays 1/512) and collapses to: conv1 -> conv2 -> squash ->
  4096->160 matvec -> elementwise squash. The convolutions are tiny, so
  cross-core collectives (a measured ~75us NRT CC setup gap per
  execution on this axon fabric) cost more than replicating them.

  * Every core computes conv1 + conv2 (PrimaryCaps) + squash redundantly:
      conv1 as a 243-contraction matmul over a host-built im2col of x;
      conv2 as 81 (dy,dx) PSUM-accumulated matmuls over strided views of
      h (no im2col materialization), weights stationary, bf16.
  * The DigitCaps matvec output (160 = 10*16) is sharded 20-per-core via
    per-core weight slices => cores are fully independent; the host just
    concatenates the 8 (1,20) results. No communication at all.
  * All PE compute in bf16 (weights host-cast), f32 PSUM/vector math.

  Schedule (from perfetto traces; 35.7us baseline -> ~33.1us):
  - the w2 stream is HBM-bound (8 cores x 5.3MB aggregate ~3TB/s =
    chip roofline). DMA engines serve all HWDGE rings, and anything
    issued behind the w2 flood starves for ~15us, so ordering is
    everything: w2 chunk 0 issues absolutely first (its data flows
    while later issues execute), then the early-needed small inputs
    (c1, biases) FIFO-ahead of the remaining chunks on the same sync
    ring; only v (first used ~3us after stream end) rides the starved
    scalar ring. Total input DMAs = 9 = the global DMA-completion-sem
    rotation depth, so no issue throttles on sem reuse. The 1-dydx
    final chunk keeps the post-stream matmul tail at 2 matmuls.
  - build_nc() strips the framework const-AP all-engine barrier from
    the main block (activations get an explicit zero-bias tile
    instead), letting the first DMA issue ~1us earlier.
  - the serial tail (squash -> digitcaps -> squash -> out DMA) is
    restructured: single memset-initialized conv2 PSUM tile +
    host-built (128,33) bias tensor (1 tensor_tensor add), Sqrt on
    Scalar overlapped with the 1+sq/reciprocal chain on Vector, u
    written in two halves so digitcaps starts early, and the final
    squash reads psum_d twice (Square on Scalar + single-PSUM-operand
    tensor_tensor) instead of copy+mul chains.
  - FastTailTileContext makes only GpSimd (the sem-clearing engine)
    wait out the DMA-completion sems; the walrus exit barrier orders
    the other engines, saving ~1us of serial waits on the exit path.

kernel(**inputs) takes the FULL unsharded inputs and returns the full
(1,1,10,16,1) float32 output.
"""
import numpy as np
import ml_dtypes

import concourse.bass as bass
import concourse.bacc as bacc
import concourse.tile as tile
import concourse.mybir as mybir
from concourse.bass_utils import run_bass_kernel_spmd
from concourse.tile import ScopedClock, add_dep_helper

FAST_TAIL = True


class FastTailTileContext(tile.TileContext):
    """TileContext tail with a 1-hop handshake instead of the all-engine
    barriers (each an EVSEM polling butterfly measured at ~7us here).

    The sync.drain waits for every tracked semaphore target, so by the
    time it passes, every sem-touching instruction on every engine has
    retired (each engine's last real work is upstream of the output DMA
    the drain waits on). A single drain->GpSimd semaphore hop then orders
    the sem/DMA-state clears; the next execution's NEFF entry barrier
    orders everything else."""

    def _drain_and_barrier(self, tick_clock, wait_clock):
        if not FAST_TAIL:
            return super()._drain_and_barrier(tick_clock, wait_clock)
        nc = self.nc
        # GpSimd (the clearing engine, otherwise idle here) waits on every
        # tracked semaphore's final value itself, then clears.
        drain_inst = nc.gpsimd.drain()
        wait_clock.add_sem_waits(
            drain_inst.ins, ScopedClock({None: tick_clock.global_clock})
        )
        # DMA completion sems update asynchronously and the final DMA (the
        # output store) has no downstream consumer, so the clock misses it:
        # wait each DMA-lane sem out to its summed final value explicitly.
        dma_totals = {}
        for insts in self.ordered_instructions_by_block.values():
            for i in insts:
                si = i.sync_info
                if si is None or not si.on_update:
                    continue
                for u in si.on_update:
                    if (u.sync_type == "semaphore" and u.update_value
                            and (u.ant_name or "").startswith("DMA")):
                        k = (u.id, u.ant_name)
                        dma_totals[k] = dma_totals.get(k, 0) + u.update_value
        # Only GpSimd (which does the range-clear below) needs to wait the
        # DMA totals: the walrus exit barrier that follows the tile block
        # already orders every other engine's postamble sem clears after
        # GpSimd's arrival. Putting these waits on all engines costs ~1us
        # of serial EVENT_SEMAPHORE time on the exit critical path.
        handles = {h.num: h for h in self.sems.allocated().values()}
        for (sid, _), tot in sorted(dma_totals.items()):
            if sid in handles:
                nc.gpsimd.wait_ge(handles[sid], tot)
        popped = nc._tile_sem_poison_stack.pop()
        assert popped is self._sem_poison
        nc.clear_and_free_semaphores(list(self.sems.allocated().values()))

BF16 = ml_dtypes.bfloat16
F32 = mybir.dt.float32
BF = mybir.dt.bfloat16

NCORES = 8
KI = 20             # digitcaps output elems per core (160 = 8*20)
# dydx positions per w2 DMA chunk. The tile framework rotates ~9
# DMA-completion sems GLOBALLY across queues; keeping the kernel's
# total input-DMA count at 9 (5 w2 + 2 c1 + cbb + v) avoids issue
# throttling on sem reuse entirely. The tiny final chunk keeps the
# post-stream matmul tail at 2 matmuls.
W2CHUNKS = [20, 20, 20, 20, 1]
assert sum(W2CHUNKS) == 81


# --------------------------------------------------------------------------
# Host-side input marshalling (pure layout transforms + dtype casts)
# --------------------------------------------------------------------------

def _host_prep(x, conv_w, conv_b, pri_w, pri_b, W):
    x = np.asarray(x, np.float32)
    conv_w = np.asarray(conv_w, np.float32)
    conv_b = np.asarray(conv_b, np.float32)
    pri_w = np.asarray(pri_w, np.float32)
    pri_b = np.asarray(pri_b, np.float32)
    W = np.asarray(W, np.float32)

    # im2col of x: (243, 256), row (c,dy,dx), col (oy*16+ox)
    im2col1 = np.empty((3, 9, 9, 16, 16), np.float32)
    for dy in range(9):
        for dx in range(9):
            im2col1[:, dy, dx] = x[0, :, dy:dy + 16, dx:dx + 16]
    im2col1 = im2col1.reshape(243, 256).astype(BF16)

    W1T = conv_w.reshape(128, 243).T.astype(BF16)  # (243, 128)

    # (ic, dydx*256 + oc2) with oc2 = cap*8 + j
    w2s = (pri_w.reshape(256, 128, 9, 9)
           .transpose(2, 3, 1, 0)          # (dy, dx, ic, oc2)
           .reshape(81, 128, 256)
           .transpose(1, 0, 2)             # (ic, dydx, oc2)
           .reshape(128, 81 * 256).astype(BF16))

    # digitcaps weights V[h, s, p, ki]:
    #   oc2 = 128h+p; cap=oc2>>3; j=oc2&7; n = cap*16 + j*2 + (s>>3); jj = s&7
    Wd = W[0]  # (512, 10, 16, 8)
    oc2 = np.arange(256)
    n_base = (oc2 >> 3) * 16 + (oc2 & 7) * 2
    V = np.empty((2, 16, 128, 160), np.float32)
    for s in range(16):
        sel = Wd[n_base + (s >> 3), :, :, s & 7]      # (256, 10, 16)
        V[:, s] = sel.reshape(2, 128, 160)

    # col 0: conv1 bias; cols 1..32: conv2 bias broadcast to [p, h*16+s]
    # (single merged DMA keeps the kernel at 9 input DMAs = the global
    # DMA-sem rotation depth, so no issue ever throttles on sem reuse).
    cbb = np.empty((128, 33), np.float32)
    cbb[:, 0] = conv_b
    cbb[:, 1:] = np.repeat(pri_b.reshape(2, 128).T, 16, axis=1)
    cbb = np.ascontiguousarray(cbb)

    c1 = np.concatenate([im2col1, W1T], axis=1)  # (243, 384) = [im2col | w1t]
    shared = {
        "c1_a": np.ascontiguousarray(c1[:128]),
        "c1_b": np.ascontiguousarray(c1[128:]),
        "w2s": w2s,
        "cbb": cbb,
    }
    per_core = []
    for c in range(NCORES):
        vsl = V[:, :, :, c * KI:(c + 1) * KI]                     # (2,16,128,20)
        vsl = vsl.transpose(2, 0, 1, 3).reshape(128, 32 * KI)     # (128, 640)
        d = dict(shared)
        d["v"] = np.ascontiguousarray(vsl).astype(BF16)
        per_core.append(d)
    return per_core


INPUT_SPECS = {
    "c1_a": ((128, 384), BF),
    "c1_b": ((115, 384), BF),
    "w2s": ((128, 81 * 256), BF),
    "v": ((128, 32 * KI), BF),
    "cbb": ((128, 33), F32),
}


# --------------------------------------------------------------------------
# Device IR
# --------------------------------------------------------------------------

def emit_kernel(tc, out_ap, ins):
    nc = tc.nc
    with (
        tc.tile_pool(name="sb", bufs=1) as sb,
        tc.tile_pool(name="ps", bufs=1, space="PSUM") as ps,
    ):
        # ---- DMA order (measured): rings share the 16 DMA engines and
        # the sync-ring w2 flood starves everything issued after it, so
        # anything needed early must be FIFO-AHEAD of the flood on the
        # sync ring itself. Chunk 0 goes absolutely first (its data
        # starts flowing while the later issues are still executing),
        # then the small early inputs, then the remaining chunks. Only v
        # (first needed by digitcaps at ~26us) can tolerate riding the
        # starved scalar ring. 8 sync + 1 scalar DMAs = the global ~9-sem
        # rotation, so no issue throttles on sem reuse.
        c1_a_sb = sb.tile([128, 384], BF)
        c1_b_sb = sb.tile([115, 384], BF)
        cbb_sb = sb.tile([128, 33], F32)
        v_sb = sb.tile([128, 32 * KI], BF)
        nc.scalar.dma_start(v_sb[:], ins["v"][:])
        w2t = []
        offs = [0]
        for cn in W2CHUNKS:
            offs.append(offs[-1] + cn)
        for j, cn in enumerate(W2CHUNKS):
            w2t.append(sb.tile([128, cn * 256], BF, name=f"w2t{j}"))
        nc.sync.dma_start(w2t[0][:], ins["w2s"][:, offs[0] * 256:offs[1] * 256])
        for t, name in ((c1_a_sb, "c1_a"), (c1_b_sb, "c1_b"), (cbb_sb, "cbb")):
            nc.sync.dma_start(t[:], ins[name][:])
        for j in range(1, len(W2CHUNKS)):
            nc.sync.dma_start(
                w2t[j][:], ins["w2s"][:, offs[j] * 256:offs[j + 1] * 256])

        # Own zero-bias tile for scalar activations: avoids the framework
        # const-AP SBUF region, which lets build_nc() drop the const-setup
        # all-engine barrier from the main block (~1us of entry latency).
        zb = sb.tile([128, 1], F32)
        nc.vector.memset(zb[:], 0.0)

        # ---- conv1: h = W1T.T @ im2col + conv_b  -> (128, 256) bf16
        psum1 = ps.tile([128, 256], F32)
        nc.tensor.matmul(psum1[:], c1_a_sb[:, 256:384], c1_a_sb[:, 0:256],
                         start=True, stop=False)
        nc.tensor.matmul(psum1[:], c1_b_sb[:, 256:384], c1_b_sb[:, 0:256],
                         start=False, stop=True)
        h_sb = sb.tile([128, 256], BF)
        nc.vector.tensor_scalar_add(h_sb[:], psum1[:], cbb_sb[:, 0:1])
        h4 = h_sb[:].rearrange("p (y x) -> p y x", y=16)

        # ---- conv2: 81 strided-view matmuls per oc2-half, PSUM-accumulated
        # into one (128, 32) tile: cols [0:16] = half 0, [16:32] = half 1.
        psum2 = ps.tile([128, 32], F32)
        # Two interleaved accumulation groups share this tile; a start=True
        # reset from one group stomps the other's first tap, so zero the
        # tile once (early, off the critical path) and accumulate without
        # start resets.
        nc.vector.memset(psum2[:], 0.0)
        chunk_of = []
        for j, cn in enumerate(W2CHUNKS):
            chunk_of += [(j, k) for k in range(cn)]
        for dydx in range(81):
            dy, dx = divmod(dydx, 9)
            j, jj = chunk_of[dydx]
            rhs = h4[:, dy:dy + 8:2, dx:dx + 8:2]
            for hh in range(2):
                nc.tensor.matmul(
                    psum2[:, hh * 16:(hh + 1) * 16],
                    w2t[j][:, jj * 256 + hh * 128: jj * 256 + (hh + 1) * 128],
                    rhs,
                    start=False, stop=(dydx == 80),
                )

        # ---- x2b = psum2 + pri_b (single tensor_tensor add)
        x2b = sb.tile([128, 32], F32)
        nc.vector.tensor_tensor(x2b[:], psum2[:], cbb_sb[:, 1:33],
                                op=mybir.AluOpType.add)

        # ---- squash factors per (p, h, s_hi) group of 8
        t2 = sb.tile([128, 32], F32)
        nc.vector.tensor_mul(t2[:], x2b[:], x2b[:])
        sq = sb.tile([128, 4], F32)
        nc.vector.tensor_reduce(
            sq[:], t2[:].rearrange("p (g e) -> p g e", e=8),
            axis=mybir.AxisListType.X, op=mybir.AluOpType.add,
        )
        # f = sqrt(sq)/512 / (1+sq)   (1/512 cij folded in)
        # (Sqrt on Scalar overlaps the 1/(1+sq) chain on Vector)
        r_ = sb.tile([128, 4], F32)
        nc.scalar.activation(
            r_[:], sq[:], mybir.ActivationFunctionType.Sqrt,
            bias=zb[:], scale=1.0 / (512.0 * 512.0),
        )
        d2 = sb.tile([128, 4], F32)
        nc.vector.tensor_scalar_add(d2[:], sq[:], 1.0)
        rec2 = sb.tile([128, 4], F32)
        nc.vector.reciprocal(rec2[:], d2[:])
        f_ = sb.tile([128, 4], F32)
        nc.vector.tensor_mul(f_[:], r_[:], rec2[:])

        # u in two halves so the digitcaps matmuls for idx 0..15 start
        # while Vector is still producing idx 16..31.
        u_sb = sb.tile([128, 32], BF)
        for hh in range(2):
            nc.vector.tensor_mul(
                u_sb[:, hh * 16:(hh + 1) * 16].rearrange(
                    "p (g e) -> p g e", e=8),
                x2b[:, hh * 16:(hh + 1) * 16].rearrange(
                    "p (g e) -> p g e", e=8),
                f_[:, hh * 2:(hh + 1) * 2].broadcast_to((128, 2, 8)),
            )

        # ---- digitcaps matvec: psum_d[0, ki] = sum_{h,s,p} u * V
        psum_d = ps.tile([1, KI], F32)
        for idx in range(32):
            nc.tensor.matmul(
                psum_d[:],
                u_sb[:, idx:idx + 1],
                v_sb[:, idx * KI:(idx + 1) * KI],
                start=(idx == 0), stop=(idx == 31),
            )

        # ---- final elementwise squash: vij = s*|s|/(1+s^2)
        # psum_d is read twice via single-PSUM-operand ops (no SBUF copy):
        # Scalar squares it; Vector multiplies it by sqrt(t3) later.
        t3 = sb.tile([1, KI], F32)
        nc.scalar.activation(t3[:], psum_d[:],
                             mybir.ActivationFunctionType.Square,
                             bias=zb[0:1, :])
        a3 = sb.tile([1, KI], F32)
        nc.scalar.activation(a3[:], t3[:], mybir.ActivationFunctionType.Sqrt,
                             bias=zb[0:1, :])
        d3 = sb.tile([1, KI], F32)
        nc.vector.tensor_scalar_add(d3[:], t3[:], 1.0)
        rec3 = sb.tile([1, KI], F32)
        nc.vector.reciprocal(rec3[:], d3[:])
        m3 = sb.tile([1, KI], F32)
        nc.vector.tensor_tensor(m3[:], psum_d[:], a3[:],
                                op=mybir.AluOpType.mult)
        o3 = sb.tile([1, KI], F32)
        nc.vector.tensor_mul(o3[:], m3[:], rec3[:])
        nc.sync.dma_start(out_ap[:], o3[:])


# --------------------------------------------------------------------------
# Build + run
# --------------------------------------------------------------------------

_CACHE = {}


def build_nc():
    nc = bacc.Bacc(
        "TRN2", target_bir_lowering=False, debug=False, num_devices=NCORES
    )
    ins = {
        name: nc.dram_tensor(name, list(shape), dt, kind="ExternalInput").ap()
        for name, (shape, dt) in INPUT_SPECS.items()
    }
    out_ap = nc.dram_tensor("out", [1, KI], F32, kind="ExternalOutput").ap()
    # Drop the const-AP setup barrier from the main block: nothing in this
    # kernel reads the framework const APs (activations get an explicit
    # in-block zero-bias tile), so the all-engine rendezvous before the
    # tile-block branch only delays the first DMA issue by ~1us.
    mb = nc.main_func.blocks[0]
    mb.instructions = [
        i for i in mb.instructions
        if not isinstance(i, (mybir.InstDrain, mybir.InstEventSemaphore))
    ]
    with FastTailTileContext(nc) as tc:
        emit_kernel(tc, out_ap, ins)
    nc.compile()
    return nc


def kernel(**inputs):
    per_core = _host_prep(**inputs)
    if "nc" not in _CACHE:
        _CACHE["nc"] = build_nc()
    res = run_bass_kernel_spmd(
        _CACHE["nc"], per_core, core_ids=list(range(NCORES))
    )
    out = np.concatenate(
        [np.asarray(res.results[c]["out"], np.float32).reshape(-1)
         for c in range(NCORES)]
    )
    return out.reshape(1, 1, 10, 16, 1)
